# revision 42
# baseline (speedup 1.0000x reference)
"""Trainium2 Bass kernel for causal multi-head attention with QKV/O projections.

Problem: x [1, 2048, 1024] f32, W_qkv [1024, 3072] (q|k|v blocks), W_o
[1024, 1024], H=16 heads, head_dim=64, dense causal attention,
y = softmax(q k^T / 8, causal) v, out = y @ W_o.

Sharding: head-parallel over 8 NeuronCores (2 heads per core). Each core
computes q/k/v projections for its 2 heads, causal attention, and a partial
O-projection (its 128 attention-output columns against its 128 rows of W_o).
The host sums the 8 partial outputs.

On-core dataflow (bf16 into the PE, f32 accumulation in PSUM):
  - xT [D, T] arrives pre-transposed from the host, so projections need no
    on-chip transposes:
       qT/kT [128, T] = W.T @ xT       (2 heads stacked on partitions)
       v     [T, 128] = x @ Wv         (lhsT = xT tiles)
    v is stored with a constant-1 column appended per head ([v_h | 1]), so
    the attention-V matmul also accumulates the softmax denominator.
  - attention is computed transposed: S_T [tk, tq] = kT-tile.T @ qT-tile,
    P_T = exp(S_T/8) in one ACT op per (tk, tq-block) position covering both
    heads (no max subtraction; |S| <= ~4 for this data), causal mask applied
    on diagonal 128x128 blocks, fully-masked blocks skipped, fully-masked
    column strips trimmed for both heads.
  - numer_T/den: [65, tq] = [v_h | 1].T @ P_T per head. The denominator row
    is broadcast across 64 partitions with a K=1 fp32 matmul against a
    column of ones, reciprocal'd on DVE, and one elementwise multiply
    produces the normalized attention output (no cross-partition reductions).
  - the normalized numer_T is exactly the O-projection lhsT: y_partial
    [T, D] = att.T.T @ wo_rows, evacuated bf16 and summed on the host.

Scheduling: the normalize + O-projection of round j are emitted AFTER the
projections of round j+1, so the PE always has independent matmul work
queued while the DVE normalize chain runs (the Tile list scheduler uses
emission order as priority).  This keeps the PE dense, which also keeps the
HAM clock gate open (2.4 GHz warm vs 1.2 GHz cold).  All input DMAs are
issued on one HWDGE queue (sync) in exact consumption order; y tiles go out
on the scalar HWDGE queue as full [128, 1024] rows.
"""

from contextlib import ExitStack

import numpy as np
import ml_dtypes

import concourse.bacc as bacc
import concourse.mybir as mybir
import concourse.tile as tile

BF16 = ml_dtypes.bfloat16
T = 2048
D = 1024
HD = 64
N_CORES = 8
KD = D // 128          # 8 contraction chunks for projections
NT128 = T // 128       # 16
NT512 = T // 512       # 4
VS = 130               # v_sb per-tile stride: [v_h0(64) | 1 | v_h1(64) | 1]
SCALE = 1.0 / 8.0      # 1/sqrt(64)

F32 = mybir.dt.float32
BF = mybir.dt.bfloat16


def _kernel(tc, y, xT, wq, wk, wv, wo, mask, ident, dbg=None):
    nc = tc.nc
    Exp = mybir.ActivationFunctionType.Exp

    with ExitStack() as ctx:
        persist = ctx.enter_context(tc.tile_pool(name="persist", bufs=1))
        ps_mm = ctx.enter_context(tc.tile_pool(name="ps_mm", bufs=2, space="PSUM"))
        ps_s = ctx.enter_context(tc.tile_pool(name="ps_s", bufs=2, space="PSUM"))
        ps_av = ctx.enter_context(tc.tile_pool(name="ps_av", bufs=1, space="PSUM"))
        pool_p = ctx.enter_context(tc.tile_pool(name="pool_p", bufs=5))
        pool_r = ctx.enter_context(tc.tile_pool(name="pool_r", bufs=2))
        pool_y = ctx.enter_context(tc.tile_pool(name="pool_y", bufs=3))

        # ---- HAM warmup first: the PE clock-gate opens only after
        # ~3.4-6.8us of sustained matmul activity.  While the input DMA
        # streams in, run dummy matmuls on memset data (no DMA dependency)
        # so the first real projection chain executes at the warm 2.4 GHz.
        warm_src = persist.tile([1, 512], BF, tag="warm")
        nc.vector.memset(warm_src[:], 0.0)
        ones_bf = persist.tile([65, HD], BF, tag="ones_bf")
        nc.vector.memset(ones_bf[:], 1.0)
        warm_ps = ps_s.tile([128, 1024], F32, tag="s")
        for w in range(12):
            nc.tensor.matmul(
                warm_ps[0:64, 0:512], lhsT=ones_bf[0:1, 0:64],
                rhs=warm_src[:], start=True, stop=True,
            )

        # ---- all input DMAs on one HWDGE queue, in consumption order ----
        wq_sb = persist.tile([128, D], BF, tag="wq")
        nc.sync.dma_start(wq_sb[:, 0:128], wq[:, 0:128])
        wk_sb = persist.tile([128, D], BF, tag="wk")
        nc.sync.dma_start(wk_sb[:, 0:128], wk[:, 0:128])
        nc.sync.dma_start(wq_sb[:, 128:D], wq[:, 128:D])
        nc.sync.dma_start(wk_sb[:, 128:D], wk[:, 128:D])

        xT_sb = persist.tile([128, KD * T], BF, tag="xT")  # d-chunk d at cols [d*T,(d+1)*T)
        xT_src = xT.rearrange("(d p) t -> p d t", p=128)
        xT_dst = xT_sb[:].rearrange("p (d t) -> p d t", t=T)
        # t-block 0 chunk-by-chunk so the first projection chain starts ASAP
        for d in range(KD):
            nc.sync.dma_start(
                xT_dst[:, d, 0:512], xT_src[:, d, 0:512]
            )
        mask_sb = persist.tile([128, 128], BF, tag="mask")
        nc.sync.dma_start(mask_sb[:], mask[:])
        wv_sb = persist.tile([128, D], BF, tag="wv")
        nc.sync.dma_start(wv_sb[:], wv[:])
        nc.sync.dma_start(xT_dst[:, :, 512:1024], xT_src[:, :, 512:1024])
        wo_sb = persist.tile([128, D], BF, tag="wo")
        nc.sync.dma_start(wo_sb[:], wo[:])
        nc.sync.dma_start(xT_dst[:, :, 1024:1536], xT_src[:, :, 1024:1536])
        nc.sync.dma_start(xT_dst[:, :, 1536:2048], xT_src[:, :, 1536:2048])

        qT_sb = persist.tile([128, T], BF, tag="qT")   # partitions 0-63 head0, 64-127 head1
        kT_sb = persist.tile([128, T], BF, tag="kT")
        v_sb = persist.tile([128, NT128 * VS], BF, tag="v")
        nc.vector.memset(v_sb[:], 1.0)                 # pre-set the ones columns
        att_sb = persist.tile([128, T], BF, tag="att")  # normalized numer_T

        def proj_qk(rnd):
            for w_sb, dst in ((wq_sb, qT_sb), (wk_sb, kT_sb)):
                ps = ps_mm.tile([128, 512], F32, tag="mm")
                for d in range(KD):
                    nc.tensor.matmul(
                        ps[:],
                        lhsT=w_sb[:, d * 128:(d + 1) * 128],
                        rhs=xT_sb[:, d * T + rnd * 512: d * T + (rnd + 1) * 512],
                        start=(d == 0), stop=(d == KD - 1),
                    )
                nc.vector.tensor_copy(dst[:, rnd * 512:(rnd + 1) * 512], ps[:])

        def proj_v(rnd):
            # two t-tiles per PSUM tile so each evacuation cast covers
            # [128, 256] (DVE per-op overhead dominates small casts).
            for t0 in range(4 * rnd, 4 * rnd + 4, 2):
                ps = ps_mm.tile([128, 512], F32, tag="mm")
                for ti in range(2):
                    t = t0 + ti
                    for d in range(KD):
                        nc.tensor.matmul(
                            ps[:, ti * 128:(ti + 1) * 128],
                            lhsT=xT_sb[:, d * T + t * 128: d * T + (t + 1) * 128],
                            rhs=wv_sb[:, d * 128:(d + 1) * 128],
                            start=(d == 0), stop=(d == KD - 1),
                        )
                # one strided cast fills v_h0 -> cols [VS*t, +64) and
                # v_h1 -> cols [VS*t+65, +64) for both tiles, leaving the
                # ones columns intact (VS = 2*65, so (t, head) folds into
                # one stride-65 dim).
                dst = v_sb[:, VS * t0: VS * (t0 + 2)].rearrange(
                    "p (m b) -> p m b", b=65)[:, :, 0:64]
                src = ps[:, 0:256].rearrange("p (m b) -> p m b", b=64)
                nc.vector.tensor_copy(dst, src)

        def attention(j):
            avden = ps_av.tile([128, 1024], F32, tag="avden")  # bank per head: [65, 512] used
            n_i = 4 * j + 4
            for i in range(n_i):
                m = i - 4 * j          # >= 0 on diagonal blocks
                off = 128 * m if m > 0 else 0
                ncol = 512 - off
                first, last = (i == 0), (i == n_i - 1)
                # both heads column-trimmed; the live region [off, 512+ncol)
                # stays contiguous so one ACT op covers it.
                s_pair = ps_s.tile([128, 1024], F32, tag="s")
                nc.tensor.matmul(
                    s_pair[:, off:512],
                    lhsT=kT_sb[0:64, i * 128:(i + 1) * 128],
                    rhs=qT_sb[0:64, j * 512 + off:(j + 1) * 512],
                    start=True, stop=True, tile_position=(0, 0),
                )
                nc.tensor.matmul(
                    s_pair[:, 512:512 + ncol],
                    lhsT=kT_sb[64:128, i * 128:(i + 1) * 128],
                    rhs=qT_sb[64:128, j * 512 + off: (j + 1) * 512],
                    start=True, stop=True, tile_position=(64, 0),
                )
                p_sb = pool_p.tile([128, 1024], BF, tag="p")
                nc.scalar.activation(
                    p_sb[:, off:512 + ncol], s_pair[:, off:512 + ncol], Exp, scale=SCALE,
                )
                if m == 0:
                    # diagonal sub-blocks of both heads are adjacent
                    # ([0,128) and [512,640)): one 3D DVE op covers both.
                    pv = p_sb[:, 0:1024].rearrange(
                        "p (r c) -> p r c", c=512)[:, :, 0:128]
                    mk = mask_sb[:][:, None, :].broadcast_to([128, 2, 128])
                    nc.vector.tensor_mul(pv, pv, mk)
                elif m > 0:
                    nc.vector.tensor_mul(
                        p_sb[:, off:off + 128],
                        p_sb[:, off:off + 128], mask_sb[:],
                    )
                    nc.vector.tensor_mul(
                        p_sb[:, 512:640], p_sb[:, 512:640], mask_sb[:],
                    )
                nc.tensor.matmul(
                    avden[0:65, off:512],
                    lhsT=v_sb[:, VS * i: VS * i + 65],
                    rhs=p_sb[:, off:512],
                    start=first, stop=last,
                )
                nc.tensor.matmul(
                    avden[0:65, 512 + off:1024],
                    lhsT=v_sb[:, VS * i + 65: VS * i + 130],
                    rhs=p_sb[:, 512:512 + ncol],
                    start=first, stop=last,
                )
            return avden

        def normalize(j, avden):
            # row 64 of each head's bank is the denominator: broadcast it
            # across 64 partitions with a cheap bf16 K=1 matmul (an f32
            # broadcast matmul runs in the slow fp32 two-pass mode), then
            # reciprocal + multiply produce the normalized attention out.
            for h in range(2):
                hc = h * 512
                denrow = pool_r.tile([65, 512], BF, tag="denrow")
                nc.vector.tensor_copy(denrow[64:65, :], avden[64:65, hc:hc + 512])
                bc_ps = ps_mm.tile([128, 512], F32, tag="mm")
                nc.tensor.matmul(
                    bc_ps[0:64, :], lhsT=ones_bf[64:65, :], rhs=denrow[64:65, :],
                    start=True, stop=True,
                )
                recip = pool_r.tile([64, 512], F32, tag="recip")
                nc.vector.reciprocal_approx_fast(recip[:], bc_ps[0:64, :])
                nc.vector.tensor_mul(
                    att_sb[h * 64:(h + 1) * 64, j * 512:(j + 1) * 512],
                    avden[0:64, hc:hc + 512], recip[:],
                )

        def oproj(j):
            for t in range(4 * j, 4 * j + 4):
                y_sb = pool_y.tile([128, 1024], BF, tag="y")
                for nh in range(2):
                    ps = ps_mm.tile([128, 512], F32, tag="mm")
                    nc.tensor.matmul(
                        ps[:],
                        lhsT=att_sb[:, t * 128:(t + 1) * 128],
                        rhs=wo_sb[:, nh * 512:(nh + 1) * 512],
                        start=True, stop=True,
                    )
                    if nh == 0:
                        nc.vector.tensor_copy(y_sb[:, 0:512], ps[:])
                    else:
                        nc.scalar.copy(y_sb[:, 512:1024], ps[:])
                    if j == NT512 - 1:
                        # tail round: ship each half as soon as it is
                        # evacuated instead of waiting for the full row
                        nc.scalar.dma_start(
                            y[t * 128:(t + 1) * 128, nh * 512:(nh + 1) * 512],
                            y_sb[:, nh * 512:(nh + 1) * 512],
                        )
                if j != NT512 - 1:
                    nc.scalar.dma_start(y[t * 128:(t + 1) * 128, :], y_sb[:])

        # ---- software-pipelined rounds.  attention(j) is emitted BEFORE
        # the next round's projections and the previous round's
        # normalize/O-projection: the Tile list scheduler uses emission
        # order as priority, so that independent matmul work becomes
        # filler for the PE whenever the exp-paced attention loop stalls,
        # spread across the whole stretch (this also keeps the HAM clock
        # gate open).
        proj_qk(0)
        proj_v(0)
        avden = None
        for j in range(NT512):
            if avden is not None:
                # must be emitted before attention(j) re-allocates the
                # single-buffered avden PSUM tile
                normalize(j - 1, avden)
            avden = attention(j)
            if j + 1 < NT512:
                proj_qk(j + 1)
                proj_v(j + 1)
            if j > 0:
                oproj(j - 1)
        normalize(NT512 - 1, avden)
        oproj(NT512 - 1)

        if dbg is not None:
            for name, sb in (("qT", qT_sb), ("kT", kT_sb), ("att", att_sb)):
                nc.sync.dma_start(dbg[name][:], sb[:])


def _build_program(debug_dumps=False):
    nc = bacc.Bacc("TRN2", debug=False, num_devices=N_CORES)
    xT = nc.dram_tensor("xT", [D, T], BF, kind="ExternalInput").ap()
    wq = nc.dram_tensor("wq", [128, D], BF, kind="ExternalInput").ap()
    wk = nc.dram_tensor("wk", [128, D], BF, kind="ExternalInput").ap()
    wv = nc.dram_tensor("wv", [128, D], BF, kind="ExternalInput").ap()
    wo = nc.dram_tensor("wo", [128, D], BF, kind="ExternalInput").ap()
    mask = nc.dram_tensor("mask", [128, 128], BF, kind="ExternalInput").ap()
    ident = nc.dram_tensor("ident", [128, 128], BF, kind="ExternalInput").ap()
    y = nc.dram_tensor("y", [T, D], BF, kind="ExternalOutput").ap()
    dbg = None
    if debug_dumps:
        dbg = {
            name: nc.dram_tensor(f"dbg_{name}", [128, T], BF, kind="ExternalOutput").ap()
            for name in ("qT", "kT", "att")
        }

    with tile.TileContext(nc) as tc:
        _kernel(tc, y, xT, wq, wk, wv, wo, mask, ident, dbg=dbg)
    nc.compile()
    return nc


_NC = None


def _get_program():
    global _NC
    if _NC is None:
        _NC = _build_program()
    return _NC


def _rearrange_w(w_cols):
    """[1024, 128] f32 slice of W_qkv -> [128, 1024] bf16 with d-chunk d at
    cols [d*128, (d+1)*128): out[p, d*128 + m] = w_cols[d*128 + p, m]."""
    return np.ascontiguousarray(
        w_cols.reshape(KD, 128, 128).transpose(1, 0, 2).reshape(128, KD * 128)
    ).astype(BF16)


def make_in_maps(x, W_qkv, W_o):
    x2 = np.asarray(x, dtype=np.float32).reshape(T, D)
    W_qkv = np.asarray(W_qkv, dtype=np.float32)
    W_o = np.asarray(W_o, dtype=np.float32)

    xT_bf = np.ascontiguousarray(x2.T).astype(BF16)
    mask = np.triu(np.ones((128, 128), dtype=np.float32)).astype(BF16)
    ident = np.eye(128, dtype=np.float32).astype(BF16)

    in_maps = []
    for c in range(N_CORES):
        cs = slice(2 * c * HD, 2 * c * HD + 128)
        in_maps.append({
            "xT": xT_bf,
            "wq": _rearrange_w(W_qkv[:, 0 * D:1 * D][:, cs]),
            "wk": _rearrange_w(W_qkv[:, 1 * D:2 * D][:, cs]),
            "wv": _rearrange_w(W_qkv[:, 2 * D:3 * D][:, cs]),
            "wo": np.ascontiguousarray(W_o[c * 128:(c + 1) * 128, :]).astype(BF16),
            "mask": mask,
            "ident": ident,
        })
    return in_maps


def combine_outputs(results):
    y_full = np.zeros((T, D), dtype=np.float32)
    for c in range(N_CORES):
        y_full += results[c]["y"].astype(np.float32)
    return y_full.reshape(1, T, D)


def kernel(x, W_qkv, W_o):
    from concourse.bass_utils import run_bass_kernel_spmd

    nc = _get_program()
    in_maps = make_in_maps(x, W_qkv, W_o)
    res = run_bass_kernel_spmd(nc, in_maps, core_ids=list(range(N_CORES)))
    return combine_outputs(res.results)


# revision 43
# speedup vs baseline: 1.0546x; 1.0546x over previous
"""Trainium2 Bass kernel for causal multi-head attention with QKV/O projections.

Problem: x [1, 2048, 1024] f32, W_qkv [1024, 3072] (q|k|v blocks), W_o
[1024, 1024], H=16 heads, head_dim=64, dense causal attention,
y = softmax(q k^T / 8, causal) v, out = y @ W_o.

Sharding: head-parallel over 8 NeuronCores (2 heads per core). Each core
computes q/k/v projections for its 2 heads, causal attention, and a partial
O-projection (its 128 attention-output columns against its 128 rows of W_o).
The host sums the 8 partial outputs.

On-core dataflow (bf16 into the PE, f32 accumulation in PSUM):
  - xT [D, T] arrives pre-transposed from the host, so projections need no
    on-chip transposes:
       qT/kT [128, T] = W.T @ xT       (2 heads stacked on partitions)
       v     [T, 128] = x @ Wv         (lhsT = xT tiles)
    v is stored with a constant-1 column appended per head ([v_h | 1]), so
    the attention-V matmul also accumulates the softmax denominator.
  - attention is computed transposed: S_T [tk, tq] = kT-tile.T @ qT-tile,
    P_T = exp(S_T/8) in one ACT op per (tk, tq-block) position covering both
    heads (no max subtraction; |S| <= ~4 for this data), causal mask applied
    on diagonal 128x128 blocks, fully-masked blocks skipped, fully-masked
    column strips trimmed for both heads.
  - numer_T/den: [65, tq] = [v_h | 1].T @ P_T per head. The denominator row
    is broadcast across 64 partitions with a K=1 fp32 matmul against a
    column of ones, reciprocal'd on DVE, and one elementwise multiply
    produces the normalized attention output (no cross-partition reductions).
  - the normalized numer_T is exactly the O-projection lhsT: y_partial
    [T, D] = att.T.T @ wo_rows, evacuated bf16 and summed on the host.

Scheduling: the normalize + O-projection of round j are emitted AFTER the
projections of round j+1, so the PE always has independent matmul work
queued while the DVE normalize chain runs (the Tile list scheduler uses
emission order as priority).  This keeps the PE dense, which also keeps the
HAM clock gate open (2.4 GHz warm vs 1.2 GHz cold).  All input DMAs are
issued on one HWDGE queue (sync) in exact consumption order; y tiles go out
on the scalar HWDGE queue as full [128, 1024] rows.
"""

from contextlib import ExitStack

import numpy as np
import ml_dtypes

import concourse.bacc as bacc
import concourse.mybir as mybir
import concourse.tile as tile

BF16 = ml_dtypes.bfloat16
T = 2048
D = 1024
HD = 64
N_CORES = 8
KD = D // 128          # 8 contraction chunks for projections
NT128 = T // 128       # 16
NT512 = T // 512       # 4
VS = 130               # v_sb per-tile stride: [v_h0(64) | 1 | v_h1(64) | 1]
SCALE = 1.0 / 8.0      # 1/sqrt(64)

F32 = mybir.dt.float32
BF = mybir.dt.bfloat16


def _kernel(tc, y, xT, wq, wk, wv, wo, mask, ident, dbg=None):
    nc = tc.nc
    Exp = mybir.ActivationFunctionType.Exp

    with ExitStack() as ctx:
        persist = ctx.enter_context(tc.tile_pool(name="persist", bufs=1))
        ps_mm = ctx.enter_context(tc.tile_pool(name="ps_mm", bufs=2, space="PSUM"))
        ps_s = ctx.enter_context(tc.tile_pool(name="ps_s", bufs=2, space="PSUM"))
        ps_av = ctx.enter_context(tc.tile_pool(name="ps_av", bufs=1, space="PSUM"))
        pool_p = ctx.enter_context(tc.tile_pool(name="pool_p", bufs=5))
        pool_r = ctx.enter_context(tc.tile_pool(name="pool_r", bufs=2))
        pool_y = ctx.enter_context(tc.tile_pool(name="pool_y", bufs=3))

        # ---- HAM warmup first: the PE clock-gate opens only after
        # ~3.4-6.8us of sustained matmul activity.  While the input DMA
        # streams in, run dummy matmuls on memset data (no DMA dependency)
        # so the first real projection chain executes at the warm 2.4 GHz.
        warm_src = persist.tile([1, 512], BF, tag="warm")
        nc.vector.memset(warm_src[:], 0.0)
        ones_bf = persist.tile([65, HD], BF, tag="ones_bf")
        nc.vector.memset(ones_bf[:], 1.0)
        warm_ps = ps_s.tile([128, 1024], F32, tag="s")
        for w in range(12):
            nc.tensor.matmul(
                warm_ps[0:64, 0:512], lhsT=ones_bf[0:1, 0:64],
                rhs=warm_src[:], start=True, stop=True,
            )

        # ---- all input DMAs on one HWDGE queue, in consumption order ----
        wq_sb = persist.tile([128, D], BF, tag="wq")
        nc.sync.dma_start(wq_sb[:, 0:128], wq[:, 0:128])
        wk_sb = persist.tile([128, D], BF, tag="wk")
        nc.sync.dma_start(wk_sb[:, 0:128], wk[:, 0:128])
        nc.sync.dma_start(wq_sb[:, 128:D], wq[:, 128:D])
        nc.sync.dma_start(wk_sb[:, 128:D], wk[:, 128:D])

        xT_sb = persist.tile([128, KD * T], BF, tag="xT")  # d-chunk d at cols [d*T,(d+1)*T)
        xT_src = xT.rearrange("(d p) t -> p d t", p=128)
        xT_dst = xT_sb[:].rearrange("p (d t) -> p d t", t=T)
        # t-block 0 chunk-by-chunk so the first projection chain starts ASAP
        for d in range(KD):
            nc.sync.dma_start(
                xT_dst[:, d, 0:512], xT_src[:, d, 0:512]
            )
        mask_sb = persist.tile([128, 128], BF, tag="mask")
        nc.sync.dma_start(mask_sb[:], mask[:])
        wv_sb = persist.tile([128, D], BF, tag="wv")
        nc.sync.dma_start(wv_sb[:], wv[:])
        nc.sync.dma_start(xT_dst[:, :, 512:1024], xT_src[:, :, 512:1024])
        wo_sb = persist.tile([128, D], BF, tag="wo")
        nc.sync.dma_start(wo_sb[:], wo[:])
        nc.sync.dma_start(xT_dst[:, :, 1024:1536], xT_src[:, :, 1024:1536])
        nc.sync.dma_start(xT_dst[:, :, 1536:2048], xT_src[:, :, 1536:2048])

        qT_sb = persist.tile([128, T], BF, tag="qT")   # partitions 0-63 head0, 64-127 head1
        kT_sb = persist.tile([128, T], BF, tag="kT")
        v_sb = persist.tile([128, NT128 * VS], BF, tag="v")
        nc.vector.memset(v_sb[:], 1.0)                 # pre-set the ones columns
        att_sb = persist.tile([128, T], BF, tag="att")  # normalized numer_T

        def proj_qk(rnd):
            for w_sb, dst in ((wq_sb, qT_sb), (wk_sb, kT_sb)):
                ps = ps_mm.tile([128, 512], F32, tag="mm")
                for d in range(KD):
                    nc.tensor.matmul(
                        ps[:],
                        lhsT=w_sb[:, d * 128:(d + 1) * 128],
                        rhs=xT_sb[:, d * T + rnd * 512: d * T + (rnd + 1) * 512],
                        start=(d == 0), stop=(d == KD - 1),
                    )
                nc.vector.tensor_copy(dst[:, rnd * 512:(rnd + 1) * 512], ps[:])

        def proj_v(rnd):
            # two t-tiles per PSUM tile so each evacuation cast covers
            # [128, 256] (DVE per-op overhead dominates small casts).
            for t0 in range(4 * rnd, 4 * rnd + 4, 2):
                ps = ps_mm.tile([128, 512], F32, tag="mm")
                for ti in range(2):
                    t = t0 + ti
                    for d in range(KD):
                        nc.tensor.matmul(
                            ps[:, ti * 128:(ti + 1) * 128],
                            lhsT=xT_sb[:, d * T + t * 128: d * T + (t + 1) * 128],
                            rhs=wv_sb[:, d * 128:(d + 1) * 128],
                            start=(d == 0), stop=(d == KD - 1),
                        )
                # one strided cast fills v_h0 -> cols [VS*t, +64) and
                # v_h1 -> cols [VS*t+65, +64) for both tiles, leaving the
                # ones columns intact (VS = 2*65, so (t, head) folds into
                # one stride-65 dim).
                dst = v_sb[:, VS * t0: VS * (t0 + 2)].rearrange(
                    "p (m b) -> p m b", b=65)[:, :, 0:64]
                src = ps[:, 0:256].rearrange("p (m b) -> p m b", b=64)
                nc.vector.tensor_copy(dst, src)

        def attention(j):
            avden = ps_av.tile([128, 1024], F32, tag="avden")  # bank per head: [65, 512] used
            n_i = 4 * j + 4
            for i in range(n_i):
                m = i - 4 * j          # >= 0 on diagonal blocks
                off = 128 * m if m > 0 else 0
                ncol = 512 - off
                first, last = (i == 0), (i == n_i - 1)
                # both heads column-trimmed; the live region [off, 512+ncol)
                # stays contiguous so one ACT op covers it.
                s_pair = ps_s.tile([128, 1024], F32, tag="s")
                nc.tensor.matmul(
                    s_pair[:, off:512],
                    lhsT=kT_sb[0:64, i * 128:(i + 1) * 128],
                    rhs=qT_sb[0:64, j * 512 + off:(j + 1) * 512],
                    start=True, stop=True, tile_position=(0, 0),
                )
                nc.tensor.matmul(
                    s_pair[:, 512:512 + ncol],
                    lhsT=kT_sb[64:128, i * 128:(i + 1) * 128],
                    rhs=qT_sb[64:128, j * 512 + off: (j + 1) * 512],
                    start=True, stop=True, tile_position=(64, 0),
                )
                p_sb = pool_p.tile([128, 1024], BF, tag="p")
                nc.scalar.activation(
                    p_sb[:, off:512 + ncol], s_pair[:, off:512 + ncol], Exp, scale=SCALE,
                )
                if m == 0:
                    # diagonal sub-blocks of both heads are adjacent
                    # ([0,128) and [512,640)): one 3D DVE op covers both.
                    pv = p_sb[:, 0:1024].rearrange(
                        "p (r c) -> p r c", c=512)[:, :, 0:128]
                    mk = mask_sb[:][:, None, :].broadcast_to([128, 2, 128])
                    nc.vector.tensor_mul(pv, pv, mk)
                elif m > 0:
                    nc.vector.tensor_mul(
                        p_sb[:, off:off + 128],
                        p_sb[:, off:off + 128], mask_sb[:],
                    )
                    nc.vector.tensor_mul(
                        p_sb[:, 512:640], p_sb[:, 512:640], mask_sb[:],
                    )
                nc.tensor.matmul(
                    avden[0:65, off:512],
                    lhsT=v_sb[:, VS * i: VS * i + 65],
                    rhs=p_sb[:, off:512],
                    start=first, stop=last,
                )
                nc.tensor.matmul(
                    avden[0:65, 512 + off:1024],
                    lhsT=v_sb[:, VS * i + 65: VS * i + 130],
                    rhs=p_sb[:, 512:512 + ncol],
                    start=first, stop=last,
                )
            return avden

        def normalize(j, avden):
            # row 64 of each head's bank is the denominator: broadcast it
            # across 64 partitions with a cheap bf16 K=1 matmul (an f32
            # broadcast matmul runs in the slow fp32 two-pass mode), then
            # reciprocal + multiply produce the normalized attention out.
            for h in range(2):
                hc = h * 512
                denrow = pool_r.tile([65, 512], BF, tag="denrow")
                nc.vector.tensor_copy(denrow[64:65, :], avden[64:65, hc:hc + 512])
                bc_ps = ps_mm.tile([128, 512], F32, tag="mm")
                nc.tensor.matmul(
                    bc_ps[0:64, :], lhsT=ones_bf[64:65, :], rhs=denrow[64:65, :],
                    start=True, stop=True,
                )
                recip = pool_r.tile([64, 512], F32, tag="recip")
                nc.vector.reciprocal_approx_fast(recip[:], bc_ps[0:64, :])
                nc.vector.tensor_mul(
                    att_sb[h * 64:(h + 1) * 64, j * 512:(j + 1) * 512],
                    avden[0:64, hc:hc + 512], recip[:],
                )

        def oproj(j):
            for t in range(4 * j, 4 * j + 4):
                y_sb = pool_y.tile([128, 1024], BF, tag="y")
                for nh in range(2):
                    ps = ps_mm.tile([128, 512], F32, tag="mm")
                    nc.tensor.matmul(
                        ps[:],
                        lhsT=att_sb[:, t * 128:(t + 1) * 128],
                        rhs=wo_sb[:, nh * 512:(nh + 1) * 512],
                        start=True, stop=True,
                    )
                    if nh == 0:
                        nc.vector.tensor_copy(y_sb[:, 0:512], ps[:])
                    else:
                        nc.scalar.copy(y_sb[:, 512:1024], ps[:])
                    if j == NT512 - 1:
                        # tail round: ship each half as soon as it is
                        # evacuated instead of waiting for the full row
                        nc.scalar.dma_start(
                            y[t * 128:(t + 1) * 128, nh * 512:(nh + 1) * 512],
                            y_sb[:, nh * 512:(nh + 1) * 512],
                        )
                if j != NT512 - 1:
                    nc.scalar.dma_start(y[t * 128:(t + 1) * 128, :], y_sb[:])

        # ---- software-pipelined rounds.  attention(j) is emitted BEFORE
        # the next round's projections and the previous round's
        # normalize/O-projection: the Tile list scheduler uses emission
        # order as priority, so that independent matmul work becomes
        # filler for the PE whenever the exp-paced attention loop stalls,
        # spread across the whole stretch (this also keeps the HAM clock
        # gate open).
        proj_qk(0)
        proj_v(0)
        avden = attention(0)
        for j in range(1, NT512):
            proj_qk(j)
            proj_v(j)
            normalize(j - 1, avden)
            oproj(j - 1)
            avden = attention(j)
        normalize(NT512 - 1, avden)
        oproj(NT512 - 1)

        if dbg is not None:
            for name, sb in (("qT", qT_sb), ("kT", kT_sb), ("att", att_sb)):
                nc.sync.dma_start(dbg[name][:], sb[:])


def _build_program(debug_dumps=False):
    nc = bacc.Bacc("TRN2", debug=False, num_devices=N_CORES)
    xT = nc.dram_tensor("xT", [D, T], BF, kind="ExternalInput").ap()
    wq = nc.dram_tensor("wq", [128, D], BF, kind="ExternalInput").ap()
    wk = nc.dram_tensor("wk", [128, D], BF, kind="ExternalInput").ap()
    wv = nc.dram_tensor("wv", [128, D], BF, kind="ExternalInput").ap()
    wo = nc.dram_tensor("wo", [128, D], BF, kind="ExternalInput").ap()
    mask = nc.dram_tensor("mask", [128, 128], BF, kind="ExternalInput").ap()
    ident = nc.dram_tensor("ident", [128, 128], BF, kind="ExternalInput").ap()
    y = nc.dram_tensor("y", [T, D], BF, kind="ExternalOutput").ap()
    dbg = None
    if debug_dumps:
        dbg = {
            name: nc.dram_tensor(f"dbg_{name}", [128, T], BF, kind="ExternalOutput").ap()
            for name in ("qT", "kT", "att")
        }

    with tile.TileContext(nc) as tc:
        _kernel(tc, y, xT, wq, wk, wv, wo, mask, ident, dbg=dbg)
    nc.compile()
    return nc


_NC = None


def _get_program():
    global _NC
    if _NC is None:
        _NC = _build_program()
    return _NC


def _rearrange_w(w_cols):
    """[1024, 128] f32 slice of W_qkv -> [128, 1024] bf16 with d-chunk d at
    cols [d*128, (d+1)*128): out[p, d*128 + m] = w_cols[d*128 + p, m]."""
    return np.ascontiguousarray(
        w_cols.reshape(KD, 128, 128).transpose(1, 0, 2).reshape(128, KD * 128)
    ).astype(BF16)


def make_in_maps(x, W_qkv, W_o):
    x2 = np.asarray(x, dtype=np.float32).reshape(T, D)
    W_qkv = np.asarray(W_qkv, dtype=np.float32)
    W_o = np.asarray(W_o, dtype=np.float32)

    xT_bf = np.ascontiguousarray(x2.T).astype(BF16)
    mask = np.triu(np.ones((128, 128), dtype=np.float32)).astype(BF16)
    ident = np.eye(128, dtype=np.float32).astype(BF16)

    in_maps = []
    for c in range(N_CORES):
        cs = slice(2 * c * HD, 2 * c * HD + 128)
        in_maps.append({
            "xT": xT_bf,
            "wq": _rearrange_w(W_qkv[:, 0 * D:1 * D][:, cs]),
            "wk": _rearrange_w(W_qkv[:, 1 * D:2 * D][:, cs]),
            "wv": _rearrange_w(W_qkv[:, 2 * D:3 * D][:, cs]),
            "wo": np.ascontiguousarray(W_o[c * 128:(c + 1) * 128, :]).astype(BF16),
            "mask": mask,
            "ident": ident,
        })
    return in_maps


def combine_outputs(results):
    y_full = np.zeros((T, D), dtype=np.float32)
    for c in range(N_CORES):
        y_full += results[c]["y"].astype(np.float32)
    return y_full.reshape(1, T, D)


def kernel(x, W_qkv, W_o):
    from concourse.bass_utils import run_bass_kernel_spmd

    nc = _get_program()
    in_maps = make_in_maps(x, W_qkv, W_o)
    res = run_bass_kernel_spmd(nc, in_maps, core_ids=list(range(N_CORES)))
    return combine_outputs(res.results)
